# revision 1
# baseline (speedup 1.0000x reference)
"""DetectionLoss kernel for Trainium2, 8 NeuronCores, data-parallel over batch.

Strategy:
  - Shard B=256 images as 32 per core.
  - Per core, on device: decode boxes, compute pairwise matching scores
    score(n,t) = relu(iw)*relu(ih) / (a1+a2)  (argmax-equivalent to IoU),
    PE-transpose score tiles to [t, n] layout, argmax over n via
    max/max_index (first-occurrence ties match jnp.argmax).
  - Losses (SmoothL1 box / CE cls / BCE conf) computed from matched
    indices; final scalar reduced on host across the 8 cores.
"""
import sys
sys.path.insert(0, "/opt/trn_rl_repo")

import numpy as np
import concourse.bass as bass
import concourse.bacc as bacc
import concourse.mybir as mybir
from concourse.bass_utils import run_bass_kernel_spmd
from concourse.tile import TileContext

F32 = mybir.dt.float32
BF16 = mybir.dt.bfloat16
U32 = mybir.dt.uint32
AF = mybir.ActivationFunctionType
OP = mybir.AluOpType

H_IMG, W_IMG = 832.0, 1472.0
B, N, T, C = 256, 1196, 64, 4
NCORES = 8
I = B // NCORES            # 32 images per core
Q = 10                     # n-chunks of 128 (1280 padded)
NP = Q * 128
LN16 = float(np.log(16.0))

_CACHE = {}


def _build():
    nc = bacc.Bacc("TRN2", target_bir_lowering=False, debug=False,
                   num_devices=NCORES)
    preds = nc.dram_tensor("preds", [I, N, 9], F32, kind="ExternalInput").ap()
    tgts = nc.dram_tensor("tgts", [I, T, 5], F32, kind="ExternalInput").ap()
    a2d = nc.dram_tensor("a2scratch", [I, T], F32)
    matched = nc.dram_tensor("matched", [I, T, 8], U32, kind="ExternalOutput").ap()

    with TileContext(nc) as tc:
        with tc.tile_pool(name="persist", bufs=1) as pp, \
             tc.tile_pool(name="work", bufs=2) as wp, \
             tc.tile_pool(name="psum", bufs=2, space="PSUM") as psp:

            # ---------------- stage A: load + decode preds ----------------
            raw = pp.tile([128, I, Q, 9], F32)
            nc.vector.memset(raw[:, :, 9, :], 0.0)
            # chunks q=0..8: preds[b, q*128+p, c] -> raw[p, b, q, c]
            for q in range(9):
                srcq = preds[:, q * 128:(q + 1) * 128, :].rearrange(
                    "b p c -> p b c")
                nc.sync.dma_start(out=raw[:, :, q, :], in_=srcq)
            # remainder chunk q=9: rows 1152..1195 -> partitions 0..43
            src9 = preds[:, 1152:1196, :].rearrange("b p c -> p b c")
            nc.sync.dma_start(out=raw[0:44, :, 9, :], in_=src9)

            P_hw = pp.tile([128, I, Q], F32)   # half width
            P_hh = pp.tile([128, I, Q], F32)
            P_cx = pp.tile([128, I, Q], F32)
            P_cy = pp.tile([128, I, Q], F32)
            P_x1 = pp.tile([128, I, Q], F32)
            P_x2 = pp.tile([128, I, Q], F32)
            P_y1 = pp.tile([128, I, Q], F32)
            P_y2 = pp.tile([128, I, Q], F32)
            P_a1 = pp.tile([128, I, Q], F32)

            ln16 = pp.tile([128, 1], F32)
            nc.gpsimd.memset(ln16[:], LN16)
            nc.scalar.activation(P_hw[:], raw[:, :, :, 2], AF.Exp, bias=ln16[:])
            nc.scalar.activation(P_hh[:], raw[:, :, :, 3], AF.Exp, bias=ln16[:])
            nc.vector.tensor_scalar(P_cx[:], raw[:, :, :, 0], W_IMG, W_IMG / 2,
                                    OP.mult, OP.subtract)
            nc.vector.tensor_scalar(P_cy[:], raw[:, :, :, 1], H_IMG, H_IMG / 2,
                                    OP.mult, OP.subtract)
            nc.vector.tensor_tensor(P_x1[:], P_cx[:], P_hw[:], OP.subtract)
            nc.vector.tensor_tensor(P_x2[:], P_cx[:], P_hw[:], OP.add)
            nc.vector.tensor_tensor(P_y1[:], P_cy[:], P_hh[:], OP.subtract)
            nc.vector.tensor_tensor(P_y2[:], P_cy[:], P_hh[:], OP.add)
            # a1 = bw*bh = 4*hw*hh
            nc.vector.tensor_tensor(P_a1[:], P_hw[:], P_hh[:], OP.mult)
            nc.vector.tensor_scalar(P_a1[:], P_a1[:], 4.0, None, OP.mult)

            # ---------------- stage B: target broadcast tiles --------------
            # B_* [128, I, T] replicated across partitions via DRAM reads
            B_x1 = pp.tile([128, I, T], F32)
            B_y1 = pp.tile([128, I, T], F32)
            B_x2 = pp.tile([128, I, T], F32)
            B_y2 = pp.tile([128, I, T], F32)
            B_a2 = pp.tile([128, I, T], F32)
            for j, bt in ((0, B_x1), (1, B_y1), (2, B_x2), (3, B_y2)):
                srcb = tgts[:, :, j].unsqueeze(0).broadcast_to([128, I, T])
                nc.sync.dma_start(out=bt[:], in_=srcb)
            # a2 in [t, b] layout, then DRAM roundtrip to broadcast
            tg_tb = pp.tile([64, I, 5], F32)
            nc.sync.dma_start(out=tg_tb[:],
                              in_=tgts[:, :, :].rearrange("b t c -> t b c"))
            a2_tb = pp.tile([64, I], F32)
            wtmp = pp.tile([64, I], F32)
            nc.vector.tensor_tensor(a2_tb[:], tg_tb[:, :, 2], tg_tb[:, :, 0],
                                    OP.subtract)
            nc.vector.tensor_tensor(wtmp[:], tg_tb[:, :, 3], tg_tb[:, :, 1],
                                    OP.subtract)
            nc.vector.tensor_tensor(a2_tb[:], a2_tb[:], wtmp[:], OP.mult)
            nc.sync.dma_start(out=a2d[:, :].rearrange("b t -> t b"),
                              in_=a2_tb[:])
            srca2 = a2d[:, :].rearrange("b t -> (b t)").unsqueeze(0) \
                             .broadcast_to([128, I * T])
            nc.sync.dma_start(out=B_a2[:].rearrange("p b t -> p (b t)"),
                              in_=srca2)

            # identity for PE transpose
            idn = pp.tile([128, 128], BF16)
            icol = pp.tile([128, 128], U32)
            irow = pp.tile([128, 128], U32)
            nc.gpsimd.iota(icol[:], pattern=[[1, 128]], base=0,
                           channel_multiplier=0)
            nc.gpsimd.iota(irow[:], pattern=[[0, 128]], base=0,
                           channel_multiplier=1)
            nc.vector.tensor_tensor(idn[:], icol[:], irow[:], OP.is_equal)

            # scores in [t-major] layout: S_T[p= i2*64+t, (pair:16, q:10, p128)]
            S_T = pp.tile([128, 16, Q, 128], BF16)

            # ---------------- stage C: pairwise scores per chunk q ---------
            for q in range(Q):
                mx = wp.tile([128, I, T], F32, tag="mx")
                Mx = wp.tile([128, I, T], F32, tag="Mx")
                iw = wp.tile([128, I, T], BF16, tag="iw")
                ih = wp.tile([128, I, T], BF16, tag="ih")
                S = wp.tile([128, I, T], F32, tag="S")
                R = wp.tile([128, I, T], BF16, tag="R")
                inter = wp.tile([128, I, T], BF16, tag="inter")
                score = wp.tile([128, I, T], BF16, tag="score")

                px2 = P_x2[:, :, q].unsqueeze(2).broadcast_to([128, I, T])
                px1 = P_x1[:, :, q].unsqueeze(2).broadcast_to([128, I, T])
                py2 = P_y2[:, :, q].unsqueeze(2).broadcast_to([128, I, T])
                py1 = P_y1[:, :, q].unsqueeze(2).broadcast_to([128, I, T])
                pa1 = P_a1[:, :, q].unsqueeze(2).broadcast_to([128, I, T])

                # engine balance: DVE does min/max + recip + bf16 muls;
                # GPSIMD (otherwise idle) takes the dense subtracts and the
                # a1+a2 add; ACT does the relus.
                my = wp.tile([128, I, T], F32, tag="mx")
                My = wp.tile([128, I, T], F32, tag="Mx")
                nc.vector.tensor_tensor(mx[:], B_x2[:], px2, OP.min)
                nc.vector.tensor_tensor(Mx[:], B_x1[:], px1, OP.max)
                nc.gpsimd.tensor_tensor(mx[:], mx[:], Mx[:], OP.subtract)
                nc.scalar.activation(iw[:], mx[:], AF.Relu)
                nc.vector.tensor_tensor(my[:], B_y2[:], py2, OP.min)
                nc.vector.tensor_tensor(My[:], B_y1[:], py1, OP.max)
                nc.gpsimd.tensor_tensor(my[:], my[:], My[:], OP.subtract)
                nc.scalar.activation(ih[:], my[:], AF.Relu)
                nc.gpsimd.tensor_tensor(S[:], B_a2[:], pa1, OP.add)
                with nc.allow_low_precision(reason="score ranking tolerates bf16"):
                    nc.vector.reciprocal(R[:], S[:])
                nc.vector.tensor_tensor(inter[:], iw[:], ih[:], OP.mult)
                nc.vector.tensor_tensor(score[:], inter[:], R[:], OP.mult)

                # transpose: per image-pair i: [128(n), 128(2 imgs x t)]
                ps = psp.tile([128, 16, 128], BF16, tag="ps")
                for i in range(16):
                    nc.tensor.transpose(
                        ps[:, i, :],
                        score[:, 2 * i:2 * i + 2, :].rearrange("p a t -> p (a t)"),
                        idn[:])
                # evacuate all pairs for this q: S_T[:, i, q, :] = ps[:, i, :]
                nc.scalar.activation(S_T[:, :, q, :], ps[:], AF.Copy)

            # ---------------- stage D: argmax over n per target ------------
            vmax = pp.tile([128, 16, 8], BF16)
            vidx = pp.tile([128, 16, 8], U32)
            for i in range(16):
                sv = S_T[:, i, :, :].rearrange("p q n -> p (q n)")
                nc.vector.max(vmax[:, i, :], sv)
                nc.vector.max_index(vidx[:, i, :], vmax[:, i, :], sv)
            # write out matched indices: row r = i2*64+t of pair i
            # matched[b, t] with b = 2*i + i2
            for i in range(16):
                for i2 in range(2):
                    nc.sync.dma_start(
                        out=matched[2 * i + i2, :, :],
                        in_=vidx[64 * i2:64 * i2 + 64, i, :])

    nc.compile()
    return nc


def kernel(predictions: np.ndarray, targets: np.ndarray) -> np.ndarray:
    import os, time
    os.environ["BASS_NEVER_TRACE"] = "1"  # no NTFF hook in this container
    predictions = np.ascontiguousarray(predictions, dtype=np.float32)
    targets = np.ascontiguousarray(targets, dtype=np.float32)
    if "nc" not in _CACHE:
        _CACHE["nc"] = _build()
    nc = _CACHE["nc"]

    in_maps = []
    for c in range(NCORES):
        sl = slice(c * I, (c + 1) * I)
        in_maps.append({"preds": predictions[sl], "tgts": targets[sl]})
    t0 = time.time()
    res = run_bass_kernel_spmd(nc, in_maps, list(range(NCORES)))
    _CACHE["last_run_ns"] = (time.time() - t0) * 1e9
    _CACHE["last_res"] = res

    matched = np.concatenate(
        [res.results[c]["matched"][:, :, 0] for c in range(NCORES)], axis=0
    ).astype(np.int64)  # (B, T)

    # ---- host-side loss finishing (cheap O(B*(N+T)) tails) ----
    p = predictions
    t = targets
    cx = (p[..., 0] * 2.0 - 1.0) * (W_IMG / 2.0)
    cy = (p[..., 1] * 2.0 - 1.0) * (H_IMG / 2.0)
    bw = np.exp(p[..., 2]) * 32.0
    bh = np.exp(p[..., 3]) * 32.0
    boxes = np.stack([cx - bw / 2, cy - bh / 2, cx + bw / 2, cy + bh / 2], -1)

    pm = np.take_along_axis(boxes, matched[:, :, None], axis=1)
    diff = pm - t[..., :4]
    ad = np.abs(diff)
    box_loss = np.where(ad < 1.0, 0.5 * diff * diff, ad - 0.5).sum()

    logits = np.take_along_axis(p[..., 5:9], matched[:, :, None], axis=1)
    lbl = t[..., 4].astype(np.int64)
    mxl = logits.max(-1, keepdims=True)
    lse = np.log(np.exp(logits - mxl).sum(-1)) + mxl[..., 0]
    picked = np.take_along_axis(logits, lbl[..., None], -1)[..., 0]
    cls_loss = (lse - picked).sum()

    pos = np.zeros((B, N), dtype=bool)
    np.put_along_axis(pos, matched, True, axis=1)
    x = p[..., 4]
    conf = (np.maximum(x, 0) - x * pos
            + np.log1p(np.exp(-np.abs(x)))).sum()

    total = (5.0 * box_loss + 1.0 * cls_loss + conf) / B
    return np.float32(total)



# revision 2
# speedup vs baseline: 3.8370x; 3.8370x over previous
"""DetectionLoss kernel for Trainium2, 8 NeuronCores, data-parallel over batch.

Strategy:
  - Shard B=256 images as 32 per core.
  - Device computes only the matching (argmax over IoU-equivalent scores);
    host finishes the three losses exactly in f32 from the matched indices.
  - Wire-traffic minimized for the axon tunnel: ship only the 4 box
    columns of predictions as f16 (2.45MB instead of 11MB), targets with
    host-precomputed area in col 4 (0.33MB), and pull back one u32 index
    per target (64KB).
  - The jitted shard_map wrapper around the Bass NEFF is built once and
    cached; per-call cost is one transfer+dispatch+fetch roundtrip.
  - Per core, on device: decode boxes, compute pairwise matching scores
    score(n,t) = relu(iw)*relu(ih) / (a1+a2)  (argmax-equivalent to IoU),
    PE-transpose score tiles to [t, n] layout, argmax over n via
    max/max_index (first-occurrence ties match jnp.argmax).
"""
import sys
sys.path.insert(0, "/opt/trn_rl_repo")
import os
os.environ.setdefault("BASS_NEVER_TRACE", "1")  # no NTFF hook in this container

import numpy as np
import concourse.bass as bass
import concourse.bacc as bacc
import concourse.mybir as mybir
import concourse.bass2jax as b2j
from concourse.tile import TileContext

F32 = mybir.dt.float32
F16 = mybir.dt.float16
BF16 = mybir.dt.bfloat16
U32 = mybir.dt.uint32
AF = mybir.ActivationFunctionType
OP = mybir.AluOpType

H_IMG, W_IMG = 832.0, 1472.0
B, N, T, C = 256, 1196, 64, 4
NCORES = 8
I = B // NCORES            # 32 images per core
Q = 10                     # n-chunks of 128 (1280 padded)
LN16 = float(np.log(16.0))

_CACHE = {}


def _build():
    nc = bacc.Bacc("TRN2", target_bir_lowering=False, debug=False,
                   num_devices=NCORES)
    preds = nc.dram_tensor("preds", [I, N, 4], F16, kind="ExternalInput").ap()
    tgts = nc.dram_tensor("tgts", [I, T, 5], F32, kind="ExternalInput").ap()
    matched = nc.dram_tensor("matched", [I, T, 1], U32, kind="ExternalOutput").ap()

    with TileContext(nc) as tc:
        with tc.tile_pool(name="persist", bufs=1) as pp, \
             tc.tile_pool(name="work", bufs=2) as wp, \
             tc.tile_pool(name="psum", bufs=2, space="PSUM") as psp:

            # ---------------- stage A: load + decode preds ----------------
            raw = pp.tile([128, I, Q, 4], F16)
            nc.vector.memset(raw[:, :, 9, :], 0.0)
            # chunks q=0..8: preds[b, q*128+p, c] -> raw[p, b, q, c]
            for q in range(9):
                srcq = preds[:, q * 128:(q + 1) * 128, :].rearrange(
                    "b p c -> p b c")
                nc.sync.dma_start(out=raw[:, :, q, :], in_=srcq)
            # remainder chunk q=9: rows 1152..1195 -> partitions 0..43
            src9 = preds[:, 1152:1196, :].rearrange("b p c -> p b c")
            nc.sync.dma_start(out=raw[0:44, :, 9, :], in_=src9)

            P_hw = pp.tile([128, I, Q], F32)   # half width
            P_hh = pp.tile([128, I, Q], F32)
            P_cx = pp.tile([128, I, Q], F32)
            P_cy = pp.tile([128, I, Q], F32)
            P_x1 = pp.tile([128, I, Q], F32)
            P_x2 = pp.tile([128, I, Q], F32)
            P_y1 = pp.tile([128, I, Q], F32)
            P_y2 = pp.tile([128, I, Q], F32)
            P_a1 = pp.tile([128, I, Q], F32)

            ln16 = pp.tile([128, 1], F32)
            nc.gpsimd.memset(ln16[:], LN16)
            nc.scalar.activation(P_hw[:], raw[:, :, :, 2], AF.Exp, bias=ln16[:])
            nc.scalar.activation(P_hh[:], raw[:, :, :, 3], AF.Exp, bias=ln16[:])
            nc.vector.tensor_scalar(P_cx[:], raw[:, :, :, 0], W_IMG, W_IMG / 2,
                                    OP.mult, OP.subtract)
            nc.vector.tensor_scalar(P_cy[:], raw[:, :, :, 1], H_IMG, H_IMG / 2,
                                    OP.mult, OP.subtract)
            nc.vector.tensor_tensor(P_x1[:], P_cx[:], P_hw[:], OP.subtract)
            nc.vector.tensor_tensor(P_x2[:], P_cx[:], P_hw[:], OP.add)
            nc.vector.tensor_tensor(P_y1[:], P_cy[:], P_hh[:], OP.subtract)
            nc.vector.tensor_tensor(P_y2[:], P_cy[:], P_hh[:], OP.add)
            # a1 = bw*bh = 4*hw*hh
            nc.vector.tensor_tensor(P_a1[:], P_hw[:], P_hh[:], OP.mult)
            nc.vector.tensor_scalar(P_a1[:], P_a1[:], 4.0, None, OP.mult)

            # ---------------- stage B: target broadcast tiles --------------
            # B_* [128, I, T] replicated across partitions via DRAM reads
            # (col 4 of tgts carries the host-precomputed target area a2)
            B_x1 = pp.tile([128, I, T], F32)
            B_y1 = pp.tile([128, I, T], F32)
            B_x2 = pp.tile([128, I, T], F32)
            B_y2 = pp.tile([128, I, T], F32)
            B_a2 = pp.tile([128, I, T], F32)
            for j, bt in ((0, B_x1), (1, B_y1), (2, B_x2), (3, B_y2),
                          (4, B_a2)):
                srcb = tgts[:, :, j].unsqueeze(0).broadcast_to([128, I, T])
                nc.sync.dma_start(out=bt[:], in_=srcb)

            # identity for PE transpose
            idn = pp.tile([128, 128], BF16)
            icol = pp.tile([128, 128], U32)
            irow = pp.tile([128, 128], U32)
            nc.gpsimd.iota(icol[:], pattern=[[1, 128]], base=0,
                           channel_multiplier=0)
            nc.gpsimd.iota(irow[:], pattern=[[0, 128]], base=0,
                           channel_multiplier=1)
            nc.vector.tensor_tensor(idn[:], icol[:], irow[:], OP.is_equal)

            # scores in [t-major] layout: S_T[p= i2*64+t, (pair:16, q:10, p128)]
            S_T = pp.tile([128, 16, Q, 128], BF16)

            # ---------------- stage C: pairwise scores per chunk q ---------
            for q in range(Q):
                mx = wp.tile([128, I, T], F32, tag="mx")
                Mx = wp.tile([128, I, T], F32, tag="Mx")
                iw = wp.tile([128, I, T], BF16, tag="iw")
                ih = wp.tile([128, I, T], BF16, tag="ih")
                S = wp.tile([128, I, T], F32, tag="S")
                R = wp.tile([128, I, T], BF16, tag="R")
                inter = wp.tile([128, I, T], BF16, tag="inter")
                score = wp.tile([128, I, T], BF16, tag="score")

                px2 = P_x2[:, :, q].unsqueeze(2).broadcast_to([128, I, T])
                px1 = P_x1[:, :, q].unsqueeze(2).broadcast_to([128, I, T])
                py2 = P_y2[:, :, q].unsqueeze(2).broadcast_to([128, I, T])
                py1 = P_y1[:, :, q].unsqueeze(2).broadcast_to([128, I, T])
                pa1 = P_a1[:, :, q].unsqueeze(2).broadcast_to([128, I, T])

                # engine balance: DVE does min/max + recip + bf16 muls;
                # GPSIMD (otherwise idle) takes the dense subtracts and the
                # a1+a2 add; ACT does the relus.
                my = wp.tile([128, I, T], F32, tag="mx")
                My = wp.tile([128, I, T], F32, tag="Mx")
                nc.vector.tensor_tensor(mx[:], B_x2[:], px2, OP.min)
                nc.vector.tensor_tensor(Mx[:], B_x1[:], px1, OP.max)
                nc.gpsimd.tensor_tensor(mx[:], mx[:], Mx[:], OP.subtract)
                nc.scalar.activation(iw[:], mx[:], AF.Relu)
                nc.vector.tensor_tensor(my[:], B_y2[:], py2, OP.min)
                nc.vector.tensor_tensor(My[:], B_y1[:], py1, OP.max)
                nc.gpsimd.tensor_tensor(my[:], my[:], My[:], OP.subtract)
                nc.scalar.activation(ih[:], my[:], AF.Relu)
                nc.gpsimd.tensor_tensor(S[:], B_a2[:], pa1, OP.add)
                with nc.allow_low_precision(reason="score ranking tolerates bf16"):
                    nc.vector.reciprocal(R[:], S[:])
                nc.vector.tensor_tensor(inter[:], iw[:], ih[:], OP.mult)
                nc.vector.tensor_tensor(score[:], inter[:], R[:], OP.mult)

                # transpose: per image-pair i: [128(n), 128(2 imgs x t)]
                ps = psp.tile([128, 16, 128], BF16, tag="ps")
                for i in range(16):
                    nc.tensor.transpose(
                        ps[:, i, :],
                        score[:, 2 * i:2 * i + 2, :].rearrange("p a t -> p (a t)"),
                        idn[:])
                # evacuate all pairs for this q: S_T[:, i, q, :] = ps[:, i, :]
                nc.scalar.activation(S_T[:, :, q, :], ps[:], AF.Copy)

            # ---------------- stage D: argmax over n per target ------------
            vmax = pp.tile([128, 16, 8], BF16)
            vidx = pp.tile([128, 16, 8], U32)
            for i in range(16):
                sv = S_T[:, i, :, :].rearrange("p q n -> p (q n)")
                nc.vector.max(vmax[:, i, :], sv)
                nc.vector.max_index(vidx[:, i, :], vmax[:, i, :], sv)
            # write out matched indices: row r = i2*64+t of pair i
            # matched[b, t] with b = 2*i + i2; only column 0 (the argmax)
            for i in range(16):
                for i2 in range(2):
                    nc.sync.dma_start(
                        out=matched[2 * i + i2, :, :],
                        in_=vidx[64 * i2:64 * i2 + 64, i, 0:1])

    nc.compile()
    return nc


def _make_call(nc):
    """Mirror run_bass_via_pjrt's multi-core branch, but build the jitted
    shard_map wrapper ONCE so warm calls skip retrace/recompile."""
    import jax
    from jax.sharding import Mesh, PartitionSpec
    from jax.experimental.shard_map import shard_map

    b2j.install_neuronx_cc_hook()

    partition_name = (nc.partition_id_tensor.name
                      if nc.partition_id_tensor else None)
    in_names, out_names, out_avals, zero_shapes = [], [], [], []
    for alloc in nc.m.functions[0].allocations:
        if not isinstance(alloc, mybir.MemoryLocationSet):
            continue
        name = alloc.memorylocations[0].name
        if alloc.kind == "ExternalInput":
            if name != partition_name:
                in_names.append(name)
        elif alloc.kind == "ExternalOutput":
            shape = tuple(alloc.tensor_shape)
            dtype = mybir.dt.np(alloc.dtype)
            out_names.append(name)
            out_avals.append(jax.core.ShapedArray(shape, dtype))
            zero_shapes.append(((NCORES * shape[0],) + shape[1:], dtype))
    n_params = len(in_names)
    n_outs = len(out_avals)
    in_names_all = list(in_names) + list(out_names)
    if partition_name is not None:
        in_names_all.append(partition_name)
    donate = tuple(range(n_params, n_params + n_outs))

    def _body(*args):
        operands = list(args)
        if partition_name is not None:
            operands.append(b2j.partition_id_tensor())
        outs = b2j._bass_exec_p.bind(
            *operands,
            out_avals=tuple(out_avals),
            in_names=tuple(in_names_all),
            out_names=tuple(out_names),
            lowering_input_output_aliases=(),
            sim_require_finite=True,
            sim_require_nnan=True,
            nc=nc,
        )
        return tuple(outs)

    devices = jax.devices()[:NCORES]
    assert len(devices) == NCORES
    mesh = Mesh(np.asarray(devices), ("core",))
    in_specs = (PartitionSpec("core"),) * (n_params + n_outs)
    out_specs = (PartitionSpec("core"),) * n_outs
    sharded = jax.jit(
        shard_map(_body, mesh=mesh, in_specs=in_specs, out_specs=out_specs,
                  check_rep=False),
        donate_argnums=donate, keep_unused=True,
    )
    return sharded, in_names, zero_shapes


def kernel(predictions: np.ndarray, targets: np.ndarray) -> np.ndarray:
    import time
    predictions = np.ascontiguousarray(predictions, dtype=np.float32)
    targets = np.ascontiguousarray(targets, dtype=np.float32)
    if "call" not in _CACHE:
        nc = _build()
        _CACHE["call"] = _make_call(nc)
    sharded, in_names, zero_shapes = _CACHE["call"]

    # ---- pack wire payload (f16 box columns; targets + host area) ----
    preds4 = np.ascontiguousarray(predictions[..., 0:4], dtype=np.float16)
    tb = targets[..., :4]
    a2 = (tb[..., 2] - tb[..., 0]) * (tb[..., 3] - tb[..., 1])
    tgts_dev = np.concatenate([tb, a2[..., None]], axis=-1)   # (B,T,5) f32
    by_name = {"preds": preds4, "tgts": tgts_dev}
    ins = [by_name[n] for n in in_names]
    zeros = [np.zeros(s, d) for s, d in zero_shapes]

    t0 = time.time()
    outs = sharded(*ins, *zeros)
    # overlap the matched-independent BCE base sum with device execution
    x = predictions[..., 4]
    conf_base = (np.maximum(x, 0.0)
                 + np.log1p(np.exp(-np.abs(x)))).sum(dtype=np.float64)
    matched = np.asarray(outs[0])[:, :, 0].astype(np.int64)   # (B,T)
    _CACHE["last_run_ns"] = (time.time() - t0) * 1e9
    _CACHE["last_matched"] = matched

    # ---- host-side exact loss finishing from gathered rows ----
    pm9 = np.take_along_axis(predictions, matched[:, :, None], axis=1)  # (B,T,9)
    cx = (pm9[..., 0] * 2.0 - 1.0) * (W_IMG / 2.0)
    cy = (pm9[..., 1] * 2.0 - 1.0) * (H_IMG / 2.0)
    hw = np.exp(pm9[..., 2]) * 16.0
    hh = np.exp(pm9[..., 3]) * 16.0
    pm = np.stack([cx - hw, cy - hh, cx + hw, cy + hh], -1)
    diff = pm - tb
    ad = np.abs(diff)
    box_loss = np.where(ad < 1.0, 0.5 * diff * diff, ad - 0.5).sum(dtype=np.float64)

    logits = pm9[..., 5:9]
    lbl = targets[..., 4].astype(np.int64)
    mxl = logits.max(-1, keepdims=True)
    lse = np.log(np.exp(logits - mxl).sum(-1)) + mxl[..., 0]
    picked = np.take_along_axis(logits, lbl[..., None], -1)[..., 0]
    cls_loss = (lse - picked).sum(dtype=np.float64)

    pos = np.zeros((B, N), dtype=bool)
    np.put_along_axis(pos, matched, True, axis=1)
    conf_loss = conf_base - x[pos].sum(dtype=np.float64)

    total = (5.0 * box_loss + 1.0 * cls_loss + conf_loss) / B
    return np.float32(total)


# revision 3
# speedup vs baseline: 4.6359x; 1.2082x over previous
"""DetectionLoss kernel for Trainium2, 8 NeuronCores, data-parallel over batch.

Strategy:
  - Shard B=256 images as 32 per core.
  - Device computes only the matching (argmax over IoU-equivalent scores);
    host finishes the three losses exactly in f32 from the matched indices.
  - Wire-traffic minimized for the axon tunnel: ship only the 4 box
    columns of predictions as f16 (2.6MB instead of 11MB), targets with
    host-precomputed area (0.33MB), and pull back one u32 index per
    target (64KB).
  - Host pre-packs both inputs in the exact SBUF layouts the kernel
    wants, so every device DMA moves fat contiguous runs (no per-element
    descriptor storms): preds as [128, I, Q, 4] f16 per core, targets
    column-major [5, I, T] f32 per core, matched out as one [128, 16]
    u32 tile per core.
  - The jitted shard_map wrapper around the Bass NEFF is built once and
    cached; per-call cost is one transfer+dispatch+fetch roundtrip.
  - Per core, on device: decode boxes, compute pairwise matching scores
    score(n,t) = relu(iw)*relu(ih) / (a1+a2)  (argmax-equivalent to IoU),
    PE-transpose score tiles to [t, n] layout, argmax over n via
    max/max_index (first-occurrence ties match jnp.argmax).
"""
import sys
sys.path.insert(0, "/opt/trn_rl_repo")
import os
os.environ.setdefault("BASS_NEVER_TRACE", "1")  # no NTFF hook in this container

import numpy as np
import concourse.bass as bass
import concourse.bacc as bacc
import concourse.mybir as mybir
import concourse.bass2jax as b2j
from concourse.tile import TileContext

F32 = mybir.dt.float32
F16 = mybir.dt.float16
BF16 = mybir.dt.bfloat16
U32 = mybir.dt.uint32
AF = mybir.ActivationFunctionType
OP = mybir.AluOpType

H_IMG, W_IMG = 832.0, 1472.0
B, N, T, C = 256, 1196, 64, 4
NCORES = 8
I = B // NCORES            # 32 images per core
Q = 10                     # n-chunks of 128 (1280 padded)
LN16 = float(np.log(16.0))

_CACHE = {}


def _build():
    nc = bacc.Bacc("TRN2", target_bir_lowering=False, debug=False,
                   num_devices=NCORES)
    # preds pre-packed on host to the SBUF layout [p, i, q, c]
    preds = nc.dram_tensor("preds", [128, I, Q, 4], F16,
                           kind="ExternalInput").ap()
    # targets column-major [5, I, T]: x1, y1, x2, y2, area
    tgts = nc.dram_tensor("tgts", [5, I, T], F32, kind="ExternalInput").ap()
    # matched[p, i] = argmax index for target (p%64) of image 2*i + p//64
    matched = nc.dram_tensor("matched", [128, 16], U32,
                             kind="ExternalOutput").ap()

    with TileContext(nc) as tc:
        with tc.tile_pool(name="persist", bufs=1) as pp, \
             tc.tile_pool(name="work", bufs=2) as wp, \
             tc.tile_pool(name="psum", bufs=2, space="PSUM") as psp:

            # ---------------- stage A: load + decode preds ----------------
            raw = pp.tile([128, I, Q, 4], F16)
            nc.sync.dma_start(out=raw[:], in_=preds[:])

            P_hw = pp.tile([128, I, Q], F32)   # half width
            P_hh = pp.tile([128, I, Q], F32)
            P_cx = pp.tile([128, I, Q], F32)
            P_cy = pp.tile([128, I, Q], F32)
            P_x1 = pp.tile([128, I, Q], F32)
            P_x2 = pp.tile([128, I, Q], F32)
            P_y1 = pp.tile([128, I, Q], F32)
            P_y2 = pp.tile([128, I, Q], F32)
            P_a1 = pp.tile([128, I, Q], F32)

            ln16 = pp.tile([128, 1], F32)
            nc.gpsimd.memset(ln16[:], LN16)
            nc.scalar.activation(P_hw[:], raw[:, :, :, 2], AF.Exp, bias=ln16[:])
            nc.scalar.activation(P_hh[:], raw[:, :, :, 3], AF.Exp, bias=ln16[:])
            nc.vector.tensor_scalar(P_cx[:], raw[:, :, :, 0], W_IMG, W_IMG / 2,
                                    OP.mult, OP.subtract)
            nc.vector.tensor_scalar(P_cy[:], raw[:, :, :, 1], H_IMG, H_IMG / 2,
                                    OP.mult, OP.subtract)
            nc.vector.tensor_tensor(P_x1[:], P_cx[:], P_hw[:], OP.subtract)
            nc.vector.tensor_tensor(P_x2[:], P_cx[:], P_hw[:], OP.add)
            nc.vector.tensor_tensor(P_y1[:], P_cy[:], P_hh[:], OP.subtract)
            nc.vector.tensor_tensor(P_y2[:], P_cy[:], P_hh[:], OP.add)
            # a1 = bw*bh = 4*hw*hh
            nc.vector.tensor_tensor(P_a1[:], P_hw[:], P_hh[:], OP.mult)
            nc.vector.tensor_scalar(P_a1[:], P_a1[:], 4.0, None, OP.mult)

            # ---------------- stage B: target broadcast tiles --------------
            # B_* [128, I, T] replicated across partitions via contiguous
            # 8KB-per-partition broadcast DMAs from the column-major dram
            B_x1 = pp.tile([128, I, T], F32)
            B_y1 = pp.tile([128, I, T], F32)
            B_x2 = pp.tile([128, I, T], F32)
            B_y2 = pp.tile([128, I, T], F32)
            B_a2 = pp.tile([128, I, T], F32)
            for j, bt in ((0, B_x1), (1, B_y1), (2, B_x2), (3, B_y2),
                          (4, B_a2)):
                srcb = tgts[j, :, :].rearrange("b t -> (b t)").unsqueeze(0) \
                                    .broadcast_to([128, I * T])
                nc.sync.dma_start(out=bt[:].rearrange("p b t -> p (b t)"),
                                  in_=srcb)

            # identity for PE transpose
            idn = pp.tile([128, 128], BF16)
            icol = pp.tile([128, 128], U32)
            irow = pp.tile([128, 128], U32)
            nc.gpsimd.iota(icol[:], pattern=[[1, 128]], base=0,
                           channel_multiplier=0)
            nc.gpsimd.iota(irow[:], pattern=[[0, 128]], base=0,
                           channel_multiplier=1)
            nc.vector.tensor_tensor(idn[:], icol[:], irow[:], OP.is_equal)

            # scores in [t-major] layout: S_T[p= i2*64+t, (pair:16, q:10, p128)]
            S_T = pp.tile([128, 16, Q, 128], BF16)

            # ---------------- stage C: pairwise scores per chunk q ---------
            for q in range(Q):
                mx = wp.tile([128, I, T], F32, tag="mx")
                Mx = wp.tile([128, I, T], F32, tag="Mx")
                iw = wp.tile([128, I, T], BF16, tag="iw")
                ih = wp.tile([128, I, T], BF16, tag="ih")
                S = wp.tile([128, I, T], F32, tag="S")
                R = wp.tile([128, I, T], BF16, tag="R")
                inter = wp.tile([128, I, T], BF16, tag="inter")
                score = wp.tile([128, I, T], BF16, tag="score")

                px2 = P_x2[:, :, q].unsqueeze(2).broadcast_to([128, I, T])
                px1 = P_x1[:, :, q].unsqueeze(2).broadcast_to([128, I, T])
                py2 = P_y2[:, :, q].unsqueeze(2).broadcast_to([128, I, T])
                py1 = P_y1[:, :, q].unsqueeze(2).broadcast_to([128, I, T])
                pa1 = P_a1[:, :, q].unsqueeze(2).broadcast_to([128, I, T])

                # engine balance: DVE does min/max + recip + bf16 muls;
                # GPSIMD (otherwise idle) takes the dense subtracts and the
                # a1+a2 add; ACT does the relus.
                my = wp.tile([128, I, T], F32, tag="mx")
                My = wp.tile([128, I, T], F32, tag="Mx")
                nc.vector.tensor_tensor(mx[:], B_x2[:], px2, OP.min)
                nc.vector.tensor_tensor(Mx[:], B_x1[:], px1, OP.max)
                nc.gpsimd.tensor_tensor(mx[:], mx[:], Mx[:], OP.subtract)
                nc.scalar.activation(iw[:], mx[:], AF.Relu)
                nc.vector.tensor_tensor(my[:], B_y2[:], py2, OP.min)
                nc.vector.tensor_tensor(My[:], B_y1[:], py1, OP.max)
                nc.gpsimd.tensor_tensor(my[:], my[:], My[:], OP.subtract)
                nc.scalar.activation(ih[:], my[:], AF.Relu)
                nc.gpsimd.tensor_tensor(S[:], B_a2[:], pa1, OP.add)
                with nc.allow_low_precision(reason="score ranking tolerates bf16"):
                    nc.vector.reciprocal(R[:], S[:])
                nc.vector.tensor_tensor(inter[:], iw[:], ih[:], OP.mult)
                nc.vector.tensor_tensor(score[:], inter[:], R[:], OP.mult)

                # transpose: per image-pair i: [128(n), 128(2 imgs x t)]
                ps = psp.tile([128, 16, 128], BF16, tag="ps")
                for i in range(16):
                    nc.tensor.transpose(
                        ps[:, i, :],
                        score[:, 2 * i:2 * i + 2, :].rearrange("p a t -> p (a t)"),
                        idn[:])
                # evacuate all pairs for this q: S_T[:, i, q, :] = ps[:, i, :]
                nc.scalar.activation(S_T[:, :, q, :], ps[:], AF.Copy)

            # ---------------- stage D: argmax over n per target ------------
            vmax = pp.tile([128, 16, 8], BF16)
            vidx = pp.tile([128, 16, 8], U32)
            for i in range(16):
                sv = S_T[:, i, :, :].rearrange("p q n -> p (q n)")
                nc.vector.max(vmax[:, i, :], sv)
                nc.vector.max_index(vidx[:, i, :], vmax[:, i, :], sv)
            # single output DMA: matched[p, i] = vidx[p, i, 0]
            nc.sync.dma_start(
                out=matched[:, :],
                in_=vidx[:, :, 0:1].rearrange("p i one -> p (i one)"))

    nc.compile()
    return nc


def _make_call(nc):
    """Mirror run_bass_via_pjrt's multi-core branch, but build the jitted
    shard_map wrapper ONCE so warm calls skip retrace/recompile."""
    import jax
    from jax.sharding import Mesh, PartitionSpec
    from jax.experimental.shard_map import shard_map

    b2j.install_neuronx_cc_hook()

    partition_name = (nc.partition_id_tensor.name
                      if nc.partition_id_tensor else None)
    in_names, out_names, out_avals, zero_shapes = [], [], [], []
    for alloc in nc.m.functions[0].allocations:
        if not isinstance(alloc, mybir.MemoryLocationSet):
            continue
        name = alloc.memorylocations[0].name
        if alloc.kind == "ExternalInput":
            if name != partition_name:
                in_names.append(name)
        elif alloc.kind == "ExternalOutput":
            shape = tuple(alloc.tensor_shape)
            dtype = mybir.dt.np(alloc.dtype)
            out_names.append(name)
            out_avals.append(jax.core.ShapedArray(shape, dtype))
            zero_shapes.append(((NCORES * shape[0],) + shape[1:], dtype))
    n_params = len(in_names)
    n_outs = len(out_avals)
    in_names_all = list(in_names) + list(out_names)
    if partition_name is not None:
        in_names_all.append(partition_name)
    donate = tuple(range(n_params, n_params + n_outs))

    def _body(*args):
        operands = list(args)
        if partition_name is not None:
            operands.append(b2j.partition_id_tensor())
        outs = b2j._bass_exec_p.bind(
            *operands,
            out_avals=tuple(out_avals),
            in_names=tuple(in_names_all),
            out_names=tuple(out_names),
            lowering_input_output_aliases=(),
            sim_require_finite=True,
            sim_require_nnan=True,
            nc=nc,
        )
        return tuple(outs)

    devices = jax.devices()[:NCORES]
    assert len(devices) == NCORES
    mesh = Mesh(np.asarray(devices), ("core",))
    in_specs = (PartitionSpec("core"),) * (n_params + n_outs)
    out_specs = (PartitionSpec("core"),) * n_outs
    sharded = jax.jit(
        shard_map(_body, mesh=mesh, in_specs=in_specs, out_specs=out_specs,
                  check_rep=False),
        donate_argnums=donate, keep_unused=True,
    )
    return sharded, in_names, zero_shapes


def _pack(predictions, targets):
    """Host-side wire packing into per-core SBUF layouts."""
    # preds: [8*128, I, Q, 4] f16 with w[c*128+p, i, q, ch] = preds[c*I+i, q*128+p, ch]
    w = np.zeros((NCORES, 128, I, Q, 4), np.float16)
    pr = predictions[:, :1152, 0:4].reshape(NCORES, I, 9, 128, 4)
    w[:, :, :, 0:9, :] = pr.transpose(0, 3, 1, 2, 4)
    pr9 = predictions[:, 1152:N, 0:4].reshape(NCORES, I, N - 1152, 4)
    w[:, 0:N - 1152, :, 9, :] = pr9.transpose(0, 2, 1, 3)
    w = w.reshape(NCORES * 128, I, Q, 4)

    # tgts: [8*5, I, T] f32 columns x1,y1,x2,y2,area
    tt = targets.reshape(NCORES, I, T, 5)
    t5 = np.empty((NCORES, 5, I, T), np.float32)
    t5[:, 0:4] = tt[..., 0:4].transpose(0, 3, 1, 2)
    t5[:, 4] = (tt[..., 2] - tt[..., 0]) * (tt[..., 3] - tt[..., 1])
    t5 = t5.reshape(NCORES * 5, I, T)
    return w, t5


def kernel(predictions: np.ndarray, targets: np.ndarray) -> np.ndarray:
    import time
    predictions = np.ascontiguousarray(predictions, dtype=np.float32)
    targets = np.ascontiguousarray(targets, dtype=np.float32)
    if "call" not in _CACHE:
        nc = _build()
        _CACHE["call"] = _make_call(nc)
    sharded, in_names, zero_shapes = _CACHE["call"]

    w, t5 = _pack(predictions, targets)
    by_name = {"preds": w, "tgts": t5}
    ins = [by_name[n] for n in in_names]
    zeros = [np.zeros(s, d) for s, d in zero_shapes]

    t0 = time.time()
    outs = sharded(*ins, *zeros)
    outs[0].copy_to_host_async()
    # overlap the matched-independent BCE base sum with device execution
    x = predictions[..., 4]
    conf_base = (np.maximum(x, 0.0)
                 + np.log1p(np.exp(-np.abs(x)))).sum(dtype=np.float64)
    m = np.asarray(outs[0])                                   # (1024, 16) u32
    _CACHE["last_run_ns"] = (time.time() - t0) * 1e9
    # m[c*128 + i2*64 + t, i] = argmax for image b = c*I + 2*i + i2, target t
    matched = (m.reshape(NCORES, 2, T, 16).transpose(0, 3, 1, 2)
               .reshape(B, T).astype(np.int64))
    _CACHE["last_matched"] = matched

    # ---- host-side exact loss finishing from gathered rows ----
    tb = targets[..., :4]
    pm9 = np.take_along_axis(predictions, matched[:, :, None], axis=1)  # (B,T,9)
    cx = (pm9[..., 0] * 2.0 - 1.0) * (W_IMG / 2.0)
    cy = (pm9[..., 1] * 2.0 - 1.0) * (H_IMG / 2.0)
    hw = np.exp(pm9[..., 2]) * 16.0
    hh = np.exp(pm9[..., 3]) * 16.0
    pm = np.stack([cx - hw, cy - hh, cx + hw, cy + hh], -1)
    diff = pm - tb
    ad = np.abs(diff)
    box_loss = np.where(ad < 1.0, 0.5 * diff * diff, ad - 0.5).sum(dtype=np.float64)

    logits = pm9[..., 5:9]
    lbl = targets[..., 4].astype(np.int64)
    mxl = logits.max(-1, keepdims=True)
    lse = np.log(np.exp(logits - mxl).sum(-1)) + mxl[..., 0]
    picked = np.take_along_axis(logits, lbl[..., None], -1)[..., 0]
    cls_loss = (lse - picked).sum(dtype=np.float64)

    pos = np.zeros((B, N), dtype=bool)
    np.put_along_axis(pos, matched, True, axis=1)
    conf_loss = conf_base - x[pos].sum(dtype=np.float64)

    total = (5.0 * box_loss + 1.0 * cls_loss + conf_loss) / B
    return np.float32(total)
